# revision 6
# baseline (speedup 1.0000x reference)
"""Llama4-style MoE (top-1 routing, 8 experts + shared SwiGLU) on 8 trn2 cores.

Sharding strategy (expert-parallel + shared-expert tensor-parallel over F):
  Core c holds expert c's weights (pre-transposed on host) and an F/8 chunk
  of the shared expert. On device, each core:
    1. computes router logits for ALL tokens (replicated, cheap),
    2. top-1 argmax + sigmoid score per token,
    3. compacts the token-ids routed to ITS expert into <=CAP slots
       (mask -> cumsum-rank -> one-hot Z matrix -> perm/valid/score via one
       PE matmul), gathers those token rows with an indirect DMA, scales by
       score, and runs the expert SwiGLU on the compacted [CAP, *] block,
    4. computes its F-chunk partial of the shared SwiGLU for all tokens.
  Outputs per core: partialT [H, T] (shared partial, transposed), routedT
  [H, CAP], meta [3, CAP] (slot -> token id, validity, score). The host sums
  the 8 shared partials and scatter-adds the routed rows.

All matmuls are fp32 (PE fp32 runs at the same per-element rate as bf16).
"""

from contextlib import ExitStack

import numpy as np

import concourse.bass as bass
import concourse.mybir as mybir
import concourse.tile as tile
from concourse import bacc
from concourse.bass import IndirectOffsetOnAxis
from concourse.bass_utils import run_bass_kernel_spmd

P = 128
T = 2048          # tokens
H = 1024          # hidden
F = 2048          # expert intermediate
E = 8             # experts == cores
FS = F // E       # shared-expert F chunk per core (256)
CAP = 384         # per-expert token capacity (mean 256, binomial tail ~1e-17)
NS = CAP // P     # slot tiles (3)
TT = T // P       # token tiles (16)
HT = H // P       # hidden tiles (8)
FT = F // P       # expert F tiles (16)
TCH = 512         # t-chunk for N<=512 fp32 matmuls
NTC = T // TCH    # 4
BIG = 1.0e6

f32 = mybir.dt.float32
i32 = mybir.dt.int32
u32 = mybir.dt.uint32
AF = mybir.ActivationFunctionType
OP = mybir.AluOpType

N_CORES = 8


def _build_program():
    nc = bacc.Bacc(
        "TRN2",
        target_bir_lowering=False,
        debug=False,
        num_devices=N_CORES,
        enable_asserts=False,
    )

    # ---- I/O ----
    x_d = nc.dram_tensor("x", [T, H], f32, kind="ExternalInput")
    xt_d = nc.dram_tensor("xT", [H, T], f32, kind="ExternalInput")
    gwt_d = nc.dram_tensor("gwT", [H, E], f32, kind="ExternalInput")
    sgt_d = nc.dram_tensor("sgT", [H, FS], f32, kind="ExternalInput")
    sut_d = nc.dram_tensor("suT", [H, FS], f32, kind="ExternalInput")
    sdt_d = nc.dram_tensor("sdT", [FS, H], f32, kind="ExternalInput")
    rgt_d = nc.dram_tensor("rgT", [H, F], f32, kind="ExternalInput")
    rut_d = nc.dram_tensor("ruT", [H, F], f32, kind="ExternalInput")
    rdt_d = nc.dram_tensor("rdT", [F, H], f32, kind="ExternalInput")
    eid_d = nc.dram_tensor("eid", [P, 1], f32, kind="ExternalInput")
    idc_d = nc.dram_tensor("idcol", [P, 1], f32, kind="ExternalInput")
    iob_d = nc.dram_tensor("iotaB", [P, CAP], f32, kind="ExternalInput")
    lsl_d = nc.dram_tensor("lsl", [TT, TT], f32, kind="ExternalInput")
    idn_d = nc.dram_tensor("ident", [P, P], f32, kind="ExternalInput")

    pt_d = nc.dram_tensor("partialT", [H, T], f32, kind="ExternalOutput")
    rt_d = nc.dram_tensor("routedT", [H, CAP], f32, kind="ExternalOutput")
    mt_d = nc.dram_tensor("meta", [3, CAP], f32, kind="ExternalOutput")

    with tile.TileContext(nc) as tc, ExitStack() as ctx:
        pp = ctx.enter_context(tc.tile_pool(name="persist", bufs=1))
        wgp = ctx.enter_context(tc.tile_pool(name="wg", bufs=2))
        wup = ctx.enter_context(tc.tile_pool(name="wu", bufs=2))
        wdp = ctx.enter_context(tc.tile_pool(name="wd", bufs=2))
        xep = ctx.enter_context(tc.tile_pool(name="xe", bufs=2))
        xsp = ctx.enter_context(tc.tile_pool(name="xs", bufs=1))
        zp = ctx.enter_context(tc.tile_pool(name="z", bufs=2))
        ocp = ctx.enter_context(tc.tile_pool(name="oc", bufs=2))
        gap = ctx.enter_context(tc.tile_pool(name="ga", bufs=2))
        smp = ctx.enter_context(tc.tile_pool(name="sm", bufs=4))
        ps_r = ctx.enter_context(tc.tile_pool(name="ps_r", bufs=1, space="PSUM"))
        ps_t = ctx.enter_context(tc.tile_pool(name="ps_t", bufs=2, space="PSUM"))
        ps_a = ctx.enter_context(tc.tile_pool(name="ps_a", bufs=2, space="PSUM"))
        ps_m = ctx.enter_context(tc.tile_pool(name="ps_m", bufs=1, space="PSUM"))
        ps_g = ctx.enter_context(tc.tile_pool(name="ps_g", bufs=2, space="PSUM"))

        # ---- persistent SBUF ----
        xt_sb = pp.tile([P, HT * T], f32)        # 64KB/part: xT h-tiles
        gw_sb = pp.tile([P, HT * E], f32)
        sg_sb = pp.tile([P, HT * FS], f32)
        su_sb = pp.tile([P, HT * FS], f32)
        sd_sb = pp.tile([P, 2 * H], f32)
        idn_sb = pp.tile([P, P], f32)
        iob_sb = pp.tile([P, CAP], f32)
        idc_sb = pp.tile([P, 1], f32)
        eid_sb = pp.tile([P, 1], f32)
        lsl_sb = pp.tile([TT, TT], f32)
        lgt_sb = pp.tile([E, T], f32, tag="bigshare")   # logitsT
        m16_sb = pp.tile([P, TT], f32)           # per-tile expert masks
        sc16_sb = pp.tile([P, TT], f32)          # per-tile scores
        z16_sb = pp.tile([TT, P], f32)           # zeros for scan
        cum_sb = pp.tile([TT, P], f32)
        rk_sb = pp.tile([TT, P], f32)            # masked global rank0
        rc_sb = pp.tile([P, TT], f32)            # rank0 back in token-tile cols
        mew_sb = pp.tile([3, CAP], f32)          # meta (perm, valid, score)
        idx_sb = pp.tile([P, NS], i32)           # gather indices per slot tile
        scc_sb = pp.tile([P, NS], f32)           # per-slot scores (columns)
        xst_sb = pp.tile([P, HT * CAP], f32)     # xs transposed, per h-tile
        ash_sb = pp.tile([P, 2 * T], f32)        # shared act, 2 f-tiles
        ar_sb = pp.tile([P, FT * CAP], f32, tag="bigshare")  # routed act

        # ---- input loads ----
        for hh in range(HT):
            nc.sync.dma_start(
                out=xt_sb[:, hh * T:(hh + 1) * T],
                in_=xt_d.ap()[hh * P:(hh + 1) * P, :],
            )
        for hh in range(HT):
            nc.sync.dma_start(
                out=gw_sb[:, hh * E:(hh + 1) * E],
                in_=gwt_d.ap()[hh * P:(hh + 1) * P, :],
            )
            nc.sync.dma_start(
                out=sg_sb[:, hh * FS:(hh + 1) * FS],
                in_=sgt_d.ap()[hh * P:(hh + 1) * P, :],
            )
            nc.sync.dma_start(
                out=su_sb[:, hh * FS:(hh + 1) * FS],
                in_=sut_d.ap()[hh * P:(hh + 1) * P, :],
            )
        for ff in range(2):
            nc.sync.dma_start(
                out=sd_sb[:, ff * H:(ff + 1) * H],
                in_=sdt_d.ap()[ff * P:(ff + 1) * P, :],
            )
        nc.sync.dma_start(out=idn_sb[:], in_=idn_d.ap()[:])
        nc.sync.dma_start(out=iob_sb[:], in_=iob_d.ap()[:])
        nc.sync.dma_start(out=idc_sb[:], in_=idc_d.ap()[:])
        nc.sync.dma_start(out=eid_sb[:], in_=eid_d.ap()[:])
        nc.sync.dma_start(out=lsl_sb[:], in_=lsl_d.ap()[:])
        nc.gpsimd.memset(z16_sb[:], 0.0)

        # ---- router: logitsT[e, t] = sum_h gwT[h, e] * xT[h, t] ----
        for tc_i in range(NTC):
            ps = ps_r.tile([E, TCH], f32, space="PSUM", tag="psr")
            for hh in range(HT):
                nc.tensor.matmul(
                    out=ps[:],
                    lhsT=gw_sb[:, hh * E:(hh + 1) * E],
                    rhs=xt_sb[:, hh * T + tc_i * TCH: hh * T + (tc_i + 1) * TCH],
                    start=(hh == 0),
                    stop=(hh == HT - 1),
                )
            nc.vector.tensor_copy(
                out=lgt_sb[:, tc_i * TCH:(tc_i + 1) * TCH], in_=ps[:]
            )

        # ---- per-token argmax / score / mask ----
        for tt in range(TT):
            trp = ps_t.tile([P, E], f32, space="PSUM", tag="pst")
            nc.tensor.transpose(
                out=trp[:],
                in_=lgt_sb[:, tt * P:(tt + 1) * P],
                identity=idn_sb[0:E, 0:E],
            )
            lg = smp.tile([P, E], f32)
            nc.vector.tensor_copy(out=lg[:], in_=trp[:])
            mx = smp.tile([P, E], f32)
            mi = smp.tile([P, E], u32)
            nc.vector.max(out=mx[:], in_=lg[:])
            nc.vector.max_index(out=mi[:], in_max=mx[:], in_values=lg[:])
            nc.scalar.activation(
                out=sc16_sb[:, tt:tt + 1], in_=mx[:, 0:1], func=AF.Sigmoid
            )
            tidf = smp.tile([P, 1], f32)
            nc.vector.tensor_copy(out=tidf[:], in_=mi[:, 0:1])
            nc.vector.tensor_tensor(
                out=m16_sb[:, tt:tt + 1], in0=tidf[:], in1=eid_sb[:],
                op=OP.is_equal,
            )

        # ---- compaction: global rank of each of my tokens ----
        mt_ps = ps_t.tile([TT, P], f32, space="PSUM", tag="pst")
        nc.tensor.transpose(out=mt_ps[:], in_=m16_sb[:], identity=idn_sb[:])
        mt16 = pp.tile([TT, P], f32)
        nc.vector.tensor_copy(out=mt16[:], in_=mt_ps[:])
        nc.vector.tensor_tensor_scan(
            out=cum_sb[:], data0=mt16[:], data1=z16_sb[:],
            initial=0.0, op0=OP.add, op1=OP.add,
        )
        off_ps = ps_t.tile([TT, 1], f32, space="PSUM", tag="pst")
        nc.tensor.matmul(
            out=off_ps[:], lhsT=lsl_sb[:], rhs=cum_sb[:, P - 1:P],
            start=True, stop=True,
        )
        off_sb = smp.tile([TT, 1], f32)
        nc.vector.tensor_copy(out=off_sb[:], in_=off_ps[:])
        # rank0_masked = cum + off - 1 + BIG*(1 - m)
        t1 = pp.tile([TT, P], f32)
        nc.vector.tensor_scalar(
            out=t1[:], in0=cum_sb[:], scalar1=off_sb[:], scalar2=BIG - 1.0,
            op0=OP.add, op1=OP.add,
        )
        t2 = pp.tile([TT, P], f32)
        nc.vector.tensor_scalar_mul(t2[:], mt16[:], BIG)
        nc.vector.tensor_tensor(
            out=rk_sb[:], in0=t1[:], in1=t2[:], op=OP.subtract
        )
        rk_ps = ps_t.tile([P, TT], f32, space="PSUM", tag="pst")
        nc.tensor.transpose(
            out=rk_ps[:], in_=rk_sb[:], identity=idn_sb[0:TT, 0:TT]
        )
        nc.vector.tensor_copy(out=rc_sb[:], in_=rk_ps[:])

        # ---- meta matmul: perm / valid / score per slot ----
        me_ps = ps_m.tile([3, CAP], f32, space="PSUM", tag="psm")
        for tt in range(TT):
            z = zp.tile([P, CAP], f32)
            nc.vector.tensor_tensor(
                out=z[:],
                in0=rc_sb[:, tt:tt + 1].to_broadcast([P, CAP]),
                in1=iob_sb[:],
                op=OP.is_equal,
            )
            l3 = smp.tile([P, 3], f32)
            nc.vector.tensor_scalar_add(l3[:, 0:1], idc_sb[:], float(tt * P))
            nc.gpsimd.memset(l3[:, 1:2], 1.0)
            nc.vector.tensor_copy(out=l3[:, 2:3], in_=sc16_sb[:, tt:tt + 1])
            nc.tensor.matmul(
                out=me_ps[:], lhsT=l3[:], rhs=z[:],
                start=(tt == 0), stop=(tt == TT - 1),
            )
        nc.vector.tensor_copy(out=mew_sb[:], in_=me_ps[:])
        nc.sync.dma_start(out=mt_d.ap()[:], in_=mew_sb[:])
        for k in range(NS):
            pc_ps = ps_t.tile([P, 3], f32, space="PSUM", tag="pst")
            nc.tensor.transpose(
                out=pc_ps[:],
                in_=mew_sb[:, k * P:(k + 1) * P],
                identity=idn_sb[0:3, 0:3],
            )
            pc = smp.tile([P, 3], f32)
            nc.vector.tensor_copy(out=pc[:], in_=pc_ps[:])
            nc.vector.tensor_copy(out=idx_sb[:, k:k + 1], in_=pc[:, 0:1])
            nc.vector.tensor_copy(out=scc_sb[:, k:k + 1], in_=pc[:, 2:3])

        # ---- gather + scale + transpose the expert's tokens ----
        for k in range(NS):
            xe = xep.tile([P, H], f32)
            nc.gpsimd.indirect_dma_start(
                out=xe[:],
                out_offset=None,
                in_=x_d.ap()[:],
                in_offset=IndirectOffsetOnAxis(ap=idx_sb[:, k:k + 1], axis=0),
            )
            xs = xsp.tile([P, H], f32)
            nc.vector.tensor_scalar_mul(xs[:], xe[:], scc_sb[:, k:k + 1])
            for hh in range(HT):
                tp = ps_t.tile([P, P], f32, space="PSUM", tag="pst")
                nc.tensor.transpose(
                    out=tp[:], in_=xs[:, hh * P:(hh + 1) * P],
                    identity=idn_sb[:],
                )
                nc.vector.tensor_copy(
                    out=xst_sb[:, hh * CAP + k * P: hh * CAP + (k + 1) * P],
                    in_=tp[:],
                )

        # ---- shared expert: G/U + act (F-chunk, all tokens) ----
        for ff in range(2):
            for tc_i in range(NTC):
                psg = ps_a.tile([P, TCH], f32, space="PSUM", tag="psa")
                for hh in range(HT):
                    nc.tensor.matmul(
                        out=psg[:],
                        lhsT=sg_sb[:, hh * FS + ff * P: hh * FS + (ff + 1) * P],
                        rhs=xt_sb[:, hh * T + tc_i * TCH: hh * T + (tc_i + 1) * TCH],
                        start=(hh == 0),
                        stop=(hh == HT - 1),
                    )
                psu = ps_a.tile([P, TCH], f32, space="PSUM", tag="psa")
                for hh in range(HT):
                    nc.tensor.matmul(
                        out=psu[:],
                        lhsT=su_sb[:, hh * FS + ff * P: hh * FS + (ff + 1) * P],
                        rhs=xt_sb[:, hh * T + tc_i * TCH: hh * T + (tc_i + 1) * TCH],
                        start=(hh == 0),
                        stop=(hh == HT - 1),
                    )
                ga = gap.tile([P, TCH], f32)
                nc.scalar.activation(out=ga[:], in_=psg[:], func=AF.Silu)
                nc.vector.tensor_tensor(
                    out=ash_sb[:, ff * T + tc_i * TCH: ff * T + (tc_i + 1) * TCH],
                    in0=ga[:], in1=psu[:], op=OP.mult,
                )

        # ---- routed expert: G/U + act on compacted tokens ----
        for ff in range(FT):
            wg = wgp.tile([P, H], f32)
            nc.sync.dma_start(
                out=wg[:].rearrange("p (a b) -> p a b", a=HT),
                in_=rgt_d.ap()[:, ff * P:(ff + 1) * P].rearrange(
                    "(a p) b -> p a b", p=P
                ),
            )
            wu = wup.tile([P, H], f32)
            nc.sync.dma_start(
                out=wu[:].rearrange("p (a b) -> p a b", a=HT),
                in_=rut_d.ap()[:, ff * P:(ff + 1) * P].rearrange(
                    "(a p) b -> p a b", p=P
                ),
            )
            psg = ps_g.tile([P, CAP], f32, space="PSUM", tag="psg")
            for hh in range(HT):
                nc.tensor.matmul(
                    out=psg[:],
                    lhsT=wg[:, hh * P:(hh + 1) * P],
                    rhs=xst_sb[:, hh * CAP:(hh + 1) * CAP],
                    start=(hh == 0),
                    stop=(hh == HT - 1),
                )
            psu = ps_g.tile([P, CAP], f32, space="PSUM", tag="psg")
            for hh in range(HT):
                nc.tensor.matmul(
                    out=psu[:],
                    lhsT=wu[:, hh * P:(hh + 1) * P],
                    rhs=xst_sb[:, hh * CAP:(hh + 1) * CAP],
                    start=(hh == 0),
                    stop=(hh == HT - 1),
                )
            ga = gap.tile([P, CAP], f32)
            nc.scalar.activation(out=ga[:], in_=psg[:], func=AF.Silu)
            nc.vector.tensor_tensor(
                out=ar_sb[:, ff * CAP:(ff + 1) * CAP],
                in0=ga[:], in1=psu[:], op=OP.mult,
            )

        # ---- shared down-proj -> partialT ----
        for hh in range(HT):
            for tc_i in range(NTC):
                ps = ps_a.tile([P, TCH], f32, space="PSUM", tag="psa")
                for ff in range(2):
                    nc.tensor.matmul(
                        out=ps[:],
                        lhsT=sd_sb[:, ff * H + hh * P: ff * H + (hh + 1) * P],
                        rhs=ash_sb[:, ff * T + tc_i * TCH: ff * T + (tc_i + 1) * TCH],
                        start=(ff == 0),
                        stop=(ff == 1),
                    )
                oc = ocp.tile([P, TCH], f32)
                nc.vector.tensor_copy(out=oc[:], in_=ps[:])
                nc.sync.dma_start(
                    out=pt_d.ap()[hh * P:(hh + 1) * P, tc_i * TCH:(tc_i + 1) * TCH],
                    in_=oc[:],
                )

        # ---- routed down-proj -> routedT ----
        for hh in range(HT):
            wd0 = wdp.tile([P, H], f32)
            nc.sync.dma_start(
                out=wd0[:].rearrange("p (a b) -> p a b", a=HT),
                in_=rdt_d.ap()[0:F // 2, hh * P:(hh + 1) * P].rearrange(
                    "(a p) b -> p a b", p=P
                ),
            )
            wd1 = wdp.tile([P, H], f32)
            nc.sync.dma_start(
                out=wd1[:].rearrange("p (a b) -> p a b", a=HT),
                in_=rdt_d.ap()[F // 2:F, hh * P:(hh + 1) * P].rearrange(
                    "(a p) b -> p a b", p=P
                ),
            )
            ps = ps_g.tile([P, CAP], f32, space="PSUM", tag="psg")
            for ff in range(FT):
                wd = wd0 if ff < 8 else wd1
                nc.tensor.matmul(
                    out=ps[:],
                    lhsT=wd[:, (ff % 8) * P:((ff % 8) + 1) * P],
                    rhs=ar_sb[:, ff * CAP:(ff + 1) * CAP],
                    start=(ff == 0),
                    stop=(ff == FT - 1),
                )
            oc = ocp.tile([P, CAP], f32)
            nc.vector.tensor_copy(out=oc[:], in_=ps[:])
            nc.sync.dma_start(
                out=rt_d.ap()[hh * P:(hh + 1) * P, :], in_=oc[:]
            )

    nc.compile()
    return nc


_PROGRAM = None


def _get_program():
    global _PROGRAM
    if _PROGRAM is None:
        _PROGRAM = _build_program()
    return _PROGRAM


def _prep_inputs(hidden_states, gate_w, shared_gate, shared_up, shared_down,
                 r_gate, r_up, r_down):
    x = np.ascontiguousarray(
        np.asarray(hidden_states, dtype=np.float32).reshape(T, H))
    xT = np.ascontiguousarray(x.T)
    gwT = np.ascontiguousarray(np.asarray(gate_w, dtype=np.float32).T)
    iotaB = np.broadcast_to(
        np.arange(CAP, dtype=np.float32)[None, :], (P, CAP)).copy()
    idcol = np.arange(P, dtype=np.float32)[:, None].copy()
    lsl = np.triu(np.ones((TT, TT), dtype=np.float32), k=1)
    ident = np.eye(P, dtype=np.float32)

    sg = np.asarray(shared_gate, dtype=np.float32)
    su = np.asarray(shared_up, dtype=np.float32)
    sd = np.asarray(shared_down, dtype=np.float32)
    rg = np.asarray(r_gate, dtype=np.float32)
    ru = np.asarray(r_up, dtype=np.float32)
    rd = np.asarray(r_down, dtype=np.float32)

    in_maps = []
    for c in range(N_CORES):
        fsl = slice(c * FS, (c + 1) * FS)
        in_maps.append({
            "x": x,
            "xT": xT,
            "gwT": gwT,
            "sgT": np.ascontiguousarray(sg[fsl, :].T),
            "suT": np.ascontiguousarray(su[fsl, :].T),
            "sdT": np.ascontiguousarray(sd[:, fsl].T),
            "rgT": np.ascontiguousarray(rg[c].T),
            "ruT": np.ascontiguousarray(ru[c].T),
            "rdT": np.ascontiguousarray(rd[c].T),
            "eid": np.full((P, 1), float(c), dtype=np.float32),
            "idcol": idcol,
            "iotaB": iotaB,
            "lsl": lsl,
            "ident": ident,
        })
    return in_maps


def kernel(hidden_states, gate_w, shared_gate, shared_up, shared_down,
           r_gate, r_up, r_down, _trace=False):
    nc = _get_program()
    in_maps = _prep_inputs(hidden_states, gate_w, shared_gate, shared_up,
                           shared_down, r_gate, r_up, r_down)
    res = run_bass_kernel_spmd(nc, in_maps, list(range(N_CORES)), trace=_trace)

    out_t = np.zeros((H, T), dtype=np.float32)
    for c in range(N_CORES):
        out_t += res.results[c]["partialT"]
    out = np.ascontiguousarray(out_t.T)

    for c in range(N_CORES):
        meta = res.results[c]["meta"]
        routed = res.results[c]["routedT"].T  # [CAP, H]
        perm = np.rint(meta[0]).astype(np.int64)
        valid = meta[1] > 0.5
        out[perm[valid]] += routed[valid]

    out = out.reshape(1, T, H)
    if _trace:
        return out, res
    return out
